# revision 32
# baseline (speedup 1.0000x reference)
"""GCN block (GCNConv + LayerNorm + ReLU) on 8 Trainium2 NeuronCores.

Strategy (matches the "shard nodes / partition edges by destination" hint):
  - out = LN(A_norm @ (x @ W^T) + b) = LN((A_norm @ x) @ W^T + b): aggregate
    raw features first (A_norm commutes with the linear map), so the random
    gather runs on node-major x and no transposes are needed anywhere.
  - Destination nodes are sharded contiguously across the 8 cores
    (6250 rows each); each core processes the edges that point into its
    shard.  x is replicated in every core's DRAM as two bf16 gather tables
    (even/odd node rows, so row indices fit dma_gather's int16 indices);
    table rows are pre-scaled by dinv[src] on the host.
  - Edge messages are bucketed per (128-dst-block, src parity) and fetched
    with one dma_gather per bucket (98 small calls overlap desc-gen across
    the 4 SWDGE queues).  Buckets are padded with trailing -1 indices and
    the per-core TRUE count is loaded into a gpsimd register (batched 8 at
    a time), so padding slots generate no DMA descriptors.  Self-loop
    messages bypass the gather entirely: each block's own rows are one
    contiguous host-packed DMA (xsl) scattered via a permuted-diagonal S.
  - For each 128-message tile the [128e x 128d] selection matrix
    S (S[e, d] = dinv[dst] if dst_e == d) is PRECOMPUTED ON HOST in
    fp8-e4m3 and streamed in with plain contiguous DMA (building S on the
    DVE dominated the v1 trace; fp8 halves its footprint and LayerNorm
    cancels the per-dst-row quantization up to the tiny bias coupling).
    The scatter-add is G_blk^T @ S accumulated in PSUM, which directly
    yields agg^T as [channel, dst] — exactly the stationary operand the
    W-matmul wants.  agg^T @ W^T gives [dst, out_ch] node-major; the bias
    is folded in as a rank-1 matmul (ones^T @ [b | sum(b)]) into the same
    PSUM tile, and LayerNorm+ReLU run on ACT/DVE: the mean comes free from
    an extra W column (row-sums), E[y^2] from one Square+accum pass, and
    the finale is a single fused Relu(y*rstd - mu*rstd) with a bf16 store.
"""

import math
import sys

sys.path.insert(0, "/opt/trn_rl_repo")

import numpy as np
import ml_dtypes

N_NODES = 50000
WIDTH = 256
N_CORES = 8
NODES_PER_CORE = N_NODES // N_CORES  # 6250
P = 128
N_BLOCKS = math.ceil(NODES_PER_CORE / P)  # 49 (last block has 106 rows)
LN_EPS = 1e-5
HALF = N_NODES // 2  # rows per gather table

USE_BF16 = True
GATHER_TILE_CAP = 8  # max tiles (128 idxs each) per dma_gather call (HW ring limit 1024)


def _preprocess(edge_index):
    """Bucket messages by (core, dst-block, src-parity table), pad each bucket
    to whole 128-edge tiles.

    Processing tile order: per block, even-table tiles then odd-table tiles.
    Gather order: even tiles of all blocks concatenated (ditto odd).
    Returns (TL, TH, dstcol[8,P,Ttot], normv[8,P,Ttot],
             idxe[8,128,8*sum(TL)] i16, idxo[8,128,8*sum(TH)] i16).
    """
    src = np.asarray(edge_index[0]).astype(np.int64)
    dst = np.asarray(edge_index[1]).astype(np.int64)
    loops = np.arange(N_NODES, dtype=np.int64)
    # degree includes self-loops; the self-loop messages themselves bypass
    # the gather (contiguous rows, see the per-block diagonal S tile below)
    deg = np.bincount(np.concatenate([dst, loops]), minlength=N_NODES).astype(
        np.float64
    )
    dinv = 1.0 / np.sqrt(deg)  # deg >= 1 thanks to self loops
    msrc = src
    mdst = dst
    norm = (dinv[msrc] * dinv[mdst]).astype(np.float32)

    core = mdst // NODES_PER_CORE
    r = mdst % NODES_PER_CORE
    blk = np.minimum(r // P, N_BLOCKS - 1)
    dcol = (r - blk * P).astype(np.float32)
    tab = msrc & 1
    gbin = (core * N_BLOCKS + blk) * 2 + tab

    order = np.argsort(gbin, kind="stable")
    msrc, norm, dcol, gbin = msrc[order], norm[order], dcol[order], gbin[order]

    cnt = np.bincount(gbin, minlength=N_CORES * N_BLOCKS * 2).reshape(
        N_CORES, N_BLOCKS, 2
    )
    TL = [int(math.ceil(int(cnt[:, b, 0].max()) / P)) for b in range(N_BLOCKS)]
    TH = [int(math.ceil(int(cnt[:, b, 1].max()) / P)) for b in range(N_BLOCKS)]
    sTL, sTH = sum(TL), sum(TH)
    Ttot = sTL + sTH + N_BLOCKS  # +1 self-loop tile per block
    # tile offsets
    EOFF = np.concatenate([[0], np.cumsum(TL)])  # even gather order
    OOFF = np.concatenate([[0], np.cumsum(TH)])  # odd gather order
    TOFF = np.concatenate(
        [[0], np.cumsum(np.asarray(TL) + np.asarray(TH) + 1)]
    )

    dstcol = np.zeros((N_CORES, P, Ttot), np.float32)
    normv = np.zeros((N_CORES, P, Ttot), np.float32)
    dval = np.zeros((N_CORES, P, Ttot), np.float32)
    idxe_flat = np.full((N_CORES, sTL * P), -1, np.int16)
    idxo_flat = np.full((N_CORES, sTH * P), -1, np.int16)

    starts = np.concatenate([[0], np.cumsum(cnt.ravel())])[:-1]
    j = np.arange(len(gbin)) - starts[gbin]  # index within bucket
    c = gbin // (N_BLOCKS * 2)
    b = (gbin // 2) % N_BLOCKS
    t = gbin & 1
    tile_in_bucket = j // P
    p = j % P
    # metadata in processing order
    tg = np.where(
        t == 0,
        TOFF[b] + tile_in_bucket,
        TOFF[b] + np.asarray(TL)[b] + tile_in_bucket,
    )
    dstcol[c, p, tg] = dcol
    normv[c, p, tg] = norm
    dval[c, p, tg] = dinv[mdst[order]].astype(np.float32)
    # self-loop tiles: partition p<64 -> local dst 2p (even global parity),
    # p>=64 -> local dst 2(p-64)+1; value dinv[global dst]
    for c2 in range(N_CORES):
        for b2 in range(N_BLOCKS):
            tgs = TOFF[b2] + TL[b2] + TH[b2]
            base = c2 * NODES_PER_CORE + b2 * P
            rows = min(P, NODES_PER_CORE - b2 * P)
            nev = (rows + 1) // 2
            nod = rows // 2
            pe_ = np.arange(nev)
            dstcol[c2, pe_, tgs] = 2 * pe_
            dval[c2, pe_, tgs] = dinv[base + 2 * pe_]
            po_ = np.arange(nod)
            dstcol[c2, 64 + po_, tgs] = 2 * po_ + 1
            dval[c2, 64 + po_, tgs] = dinv[base + 2 * po_ + 1]
    # gather index arrays (per-table tile order)
    idx16 = (msrc >> 1).astype(np.int16)
    Je = (EOFF[b] + tile_in_bucket) * P + p
    Jo = (OOFF[b] + tile_in_bucket) * P + p
    ev = t == 0
    idxe_flat[c[ev], Je[ev]] = idx16[ev]
    idxo_flat[c[~ev], Jo[~ev]] = idx16[~ev]

    # merged gather calls: pairs of adjacent blocks share one call per parity.
    # The first block's pad slots must hold a VALID index (0) since only
    # trailing negatives are elided; the second block's pads stay -1.
    pairs = _pairs(TL, TH)
    ncalls = len(pairs) * 2
    ccnt = np.zeros((N_CORES, ncalls), np.int32)
    for pi, pr in enumerate(pairs):
        a = pr[0]
        if len(pr) == 2:
            b2 = pr[1]
            # zero-fill block a's pads in both parity streams
            for c2 in range(N_CORES):
                e_lo = EOFF[a] * P + cnt[c2, a, 0]
                idxe_flat[c2, e_lo : (EOFF[a] + TL[a]) * P] = 0
                o_lo = OOFF[a] * P + cnt[c2, a, 1]
                idxo_flat[c2, o_lo : (OOFF[a] + TH[a]) * P] = 0
            ccnt[:, 2 * pi] = TL[a] * P + cnt[:, b2, 0]
            ccnt[:, 2 * pi + 1] = TH[a] * P + cnt[:, b2, 1]
        else:
            ccnt[:, 2 * pi] = cnt[:, a, 0]
            ccnt[:, 2 * pi + 1] = cnt[:, a, 1]

    # wrap: flat j -> (partition j%16, column j//16), replicated on 8 stripes
    def wrap(flat, ntiles):
        if ntiles == 0:
            return np.zeros((N_CORES, P, 0), np.int16)
        a = flat.reshape(N_CORES, ntiles * 8, 16).transpose(0, 2, 1)  # [8,16,cols]
        return np.ascontiguousarray(np.tile(a, (1, 8, 1)))  # [8,128,cols]

    return (TL, TH, dstcol, normv, dval, dinv.astype(np.float32),
            wrap(idxe_flat, sTL), wrap(idxo_flat, sTH), ccnt)


def _pairs(TL, TH):
    """Greedy pairing of adjacent blocks into merged gather calls; each
    parity's tile total must stay within the 1024-descriptor ring cap."""
    return [(b,) for b in range(N_BLOCKS)]


def _chunks(TL, TH):
    return [
        (
            list(pr),
            sum(TL[b] for b in pr),
            sum(TH[b] for b in pr),
        )
        for pr in _pairs(TL, TH)
    ]


def _build_program(TL, TH, generic_affine):
    import concourse.bass as bass
    import concourse.tile as tile
    from concourse import bacc as bacc_mod
    from concourse import mybir
    from contextlib import ExitStack

    f32 = mybir.dt.float32
    bf16 = mybir.dt.bfloat16
    cdt = bf16 if USE_BF16 else f32
    i16 = mybir.dt.int16
    Alu = mybir.AluOpType
    Act = mybir.ActivationFunctionType
    sTL, sTH = sum(TL), sum(TH)
    Ttot = sTL + sTH + N_BLOCKS
    EOFF = np.concatenate([[0], np.cumsum(TL)])
    OOFF = np.concatenate([[0], np.cumsum(TH)])
    TOFF = np.concatenate([[0], np.cumsum(np.asarray(TL) + np.asarray(TH) + 1)])
    chunks = _chunks(TL, TH)
    max_nt = max(ch[1] + ch[2] for ch in chunks)

    nc = bacc_mod.Bacc(None, target_bir_lowering=False, debug=False, num_swdge_queues=4)
    xe_d = nc.declare_dram_parameter("xe", [HALF, WIDTH], cdt, isOutput=False)
    xo_d = nc.declare_dram_parameter("xo", [HALF, WIDTH], cdt, isOutput=False)
    xsl_d = nc.declare_dram_parameter("xsl", [N_BLOCKS * P, WIDTH], cdt, isOutput=False)
    idxe_d = nc.declare_dram_parameter("idxe", [P, 8 * sTL], i16, isOutput=False)
    idxo_d = nc.declare_dram_parameter("idxo", [P, 8 * sTH], i16, isOutput=False)
    f8 = mybir.dt.float8e4
    smat_d = nc.declare_dram_parameter("smat", [P, Ttot * P], f8, isOutput=False)
    wt_d = nc.declare_dram_parameter("wt", [P, 2 * (WIDTH + 1)], cdt, isOutput=False)
    be_d = nc.declare_dram_parameter("be", [1, WIDTH + 1], cdt, isOutput=False)
    i32 = mybir.dt.int32
    NCALLS = 2 * len(_pairs(TL, TH))
    cnts_d = nc.declare_dram_parameter("cnts", [1, NCALLS], i32, isOutput=False)
    if generic_affine:
        gb_d = nc.declare_dram_parameter("gb", [P, 2 * WIDTH], f32, isOutput=False)
    out_d = nc.declare_dram_parameter("out", [NODES_PER_CORE, WIDTH], cdt, isOutput=True)

    with tile.TileContext(nc) as tc:
        with ExitStack() as ctx:
            const = ctx.enter_context(tc.tile_pool(name="const", bufs=1))
            gpool = ctx.enter_context(tc.tile_pool(name="g", bufs=8))
            gspool = ctx.enter_context(tc.tile_pool(name="gs", bufs=4))
            spool = ctx.enter_context(tc.tile_pool(name="s", bufs=4))
            apool = ctx.enter_context(tc.tile_pool(name="aggT", bufs=3))
            ypool = ctx.enter_context(tc.tile_pool(name="y", bufs=3))
            stat = ctx.enter_context(tc.tile_pool(name="stat", bufs=4))
            ppool = ctx.enter_context(tc.tile_pool(name="psA", bufs=3, space="PSUM"))
            opsum = ctx.enter_context(tc.tile_pool(name="psO", bufs=2, space="PSUM"))

            cnts_sb = const.tile([1, NCALLS], i32)
            nc.sync.dma_start(cnts_sb[:], cnts_d[:, :])
            idxe_sb = const.tile([P, 8 * sTL], i16)
            nc.sync.dma_start(idxe_sb[:], idxe_d[:, :])
            idxo_sb = const.tile([P, 8 * sTH], i16)
            nc.sync.dma_start(idxo_sb[:], idxo_d[:, :])
            wt_sb = const.tile([P, 2 * (WIDTH + 1)], cdt)
            nc.sync.dma_start(wt_sb[:], wt_d[:, :])
            be_sb = const.tile([1, WIDTH + 1], cdt)
            nc.sync.dma_start(be_sb[:], be_d[:, :])
            ones_sb = const.tile([1, P], cdt)
            nc.vector.memset(ones_sb[:], 1.0)
            eps_sb = const.tile([P, 1], f32)
            nc.vector.memset(eps_sb[:], LN_EPS)
            if generic_affine:
                gb_sb = const.tile([P, 2 * WIDTH], f32)
                nc.sync.dma_start(gb_sb[:], gb_d[:, :])
                gamma_sb = gb_sb[:, 0:WIDTH]
                beta_sb = gb_sb[:, WIDTH : 2 * WIDTH]

            # warm up the Q7 gather path while the real idx tables load
            wi = const.tile([P, 8], i16)
            nc.vector.memset(wi[:], 0)
            wg = const.tile([P, 1, WIDTH], cdt)
            nc.gpsimd.dma_gather(
                wg[:], xe_d[:, :], wi[:, :], P, P, WIDTH, queue_num=3
            )

            qn = 0
            gregs = [nc.gpsimd.alloc_register(f"gcnt{i}") for i in range(8)]
            GP_BUFS = 8
            # zero all gather buffers once: rows beyond a bucket's true count
            # are never written by the gather and must not be NaN (S is 0
            # there, but 0*NaN would poison the PSUM accumulation)
            maxTL = max(ch[1] for ch in chunks)
            maxTH = max(ch[2] for ch in chunks)
            for _ in range(GP_BUFS):
                zt = gpool.tile([P, maxTL, WIDTH], cdt, tag="ge")
                nc.vector.memset(zt[:], 0.0)
                zt2 = gpool.tile([P, maxTH, WIDTH], cdt, tag="go")
                nc.vector.memset(zt2[:], 0.0)

            def bucket_gather(tag, ntiles, idx_sb, off, x_d, cidx):
                if ntiles == 0:
                    return None
                nonlocal qn
                gt = gpool.tile([P, ntiles, WIDTH], cdt, tag=tag)
                assert cidx == qn, (cidx, qn)  # batched reg loads rely on this
                if qn % 8 == 0:
                    hi = min(qn + 8, NCALLS)
                    nc.gpsimd.reg_load(gregs[: hi - qn], cnts_sb[0:1, qn:hi])
                reg = gregs[qn % 8]
                nc.gpsimd.dma_gather(
                    gt[:],
                    x_d[:, :],
                    idx_sb[:, 8 * off : 8 * (off + ntiles)],
                    ntiles * P,
                    reg,
                    WIDTH,
                    single_packet=False,
                    queue_num=qn % 4,
                )
                qn += 1
                return gt

            for pi, (blocks, ne, no) in enumerate(chunks):
                tgc0 = int(TOFF[blocks[0]])
                nt_chunk = ne + no + len(blocks)
                s_sb = spool.tile([P, nt_chunk * P], f8, tag="schunk")
                nc.sync.dma_start(s_sb[:], smat_d[:, tgc0 * P : (tgc0 + nt_chunk) * P])
                a = blocks[0]
                ge_m = bucket_gather("ge", ne, idxe_sb, int(EOFF[a]), xe_d, 2 * pi)
                go_m = bucket_gather("go", no, idxo_sb, int(OOFF[a]), xo_d, 2 * pi + 1)
                gstiles = {}
                for b in blocks:
                    # self-loop rows, host-packed per block: partitions 0-63 =
                    # even-parity dsts, 64-127 = odd (zeros in pad rows)
                    gs = gspool.tile([P, 1, WIDTH], cdt, tag="gs")
                    nc.sync.dma_start(gs[:, 0, :], xsl_d[b * P : (b + 1) * P, :])
                    gstiles[b] = gs
                for b in blocks:
                    tg0 = int(TOFF[b])
                    gs = gstiles[b]
                    eoff = int(EOFF[b] - EOFF[a])
                    ooff = int(OOFF[b] - OOFF[a])
                    seq = [(ge_m, eoff + t) for t in range(TL[b])] + [
                        (go_m, ooff + t) for t in range(TH[b])
                    ] + [(gs, 0)]
                    nt = len(seq)
                    ps0_t = ppool.tile([P, P], f32, tag="ps0")
                    ps1_t = ppool.tile([P, P], f32, tag="ps1")
                    ps0 = ps0_t[:]
                    ps1 = ps1_t[:]
                    for k, (gt, col) in enumerate(seq):
                        so = (tg0 - tgc0 + k) * P
                        s_ap = s_sb[:, so : so + P]
                        nc.tensor.matmul(
                            out=ps0,
                            lhsT=gt[:, col, 0:P],
                            rhs=s_ap,
                            start=(k == 0),
                            stop=(k == nt - 1),
                            skip_group_check=True,
                        )
                        nc.tensor.matmul(
                            out=ps1,
                            lhsT=gt[:, col, P:WIDTH],
                            rhs=s_ap,
                            start=(k == 0),
                            stop=(k == nt - 1),
                            skip_group_check=True,
                        )
                    # aggT blocks [128 ch, 128 dst] -> SBUF (cast) for W-matmul
                    a0 = apool.tile([P, P], cdt, tag="a0")
                    nc.scalar.copy(a0[:], ps0)
                    a1 = apool.tile([P, P], cdt, tag="a1")
                    nc.scalar.copy(a1[:], ps1)
                    po = opsum.tile([P, WIDTH + 1], f32, tag="po")
                    nc.tensor.matmul(
                        out=po[:],
                        lhsT=a0[:],
                        rhs=wt_sb[:, : WIDTH + 1],
                        start=True,
                        stop=False,
                    )
                    nc.tensor.matmul(
                        out=po[:],
                        lhsT=a1[:],
                        rhs=wt_sb[:, WIDTH + 1 :],
                        start=False,
                        stop=False,
                    )
                    # rank-1 bias add: po += ones^T @ [b | sum(b)]
                    nc.tensor.matmul(
                        out=po[:],
                        lhsT=ones_sb[:, :],
                        rhs=be_sb[:, :],
                        start=False,
                        stop=True,
                        skip_group_check=True,
                    )
                    # ---- epilogue: LayerNorm stats + fused scale/ReLU ----
                    # po[:, :256] == y (bias already added); po[:, 256] == 256*mean(y)
                    sq = ypool.tile([P, WIDTH], cdt, tag="sq")
                    ey2 = stat.tile([P, 1], f32, tag="ey2")
                    nc.scalar.activation(
                        out=sq[:],
                        in_=po[:, :WIDTH],
                        func=Act.Square,
                        scale=1.0 / 16.0,
                        accum_out=ey2[:],
                    )
                    mu = stat.tile([P, 1], f32, tag="mu")
                    nc.vector.tensor_scalar_mul(mu[:], po[:, WIDTH : WIDTH + 1], 1.0 / WIDTH)
                    m2 = stat.tile([P, 1], f32, tag="m2")
                    nc.vector.tensor_tensor(m2[:], mu[:], mu[:, 0:1], Alu.mult)
                    var = stat.tile([P, 1], f32, tag="var")
                    nc.vector.tensor_scalar_sub(var[:], ey2[:], m2[:, 0:1])
                    sd = stat.tile([P, 1], f32, tag="sd")
                    nc.scalar.activation(
                        out=sd[:], in_=var[:], func=Act.Sqrt, bias=eps_sb[:, :1]
                    )
                    rstd = stat.tile([P, 1], f32, tag="rstd")
                    nc.vector.reciprocal(rstd[:], sd[:])
                    nb = stat.tile([P, 1], f32, tag="nb")
                    nc.vector.tensor_scalar(
                        out=nb[:],
                        in0=mu[:],
                        scalar1=rstd[:, 0:1],
                        scalar2=-1.0,
                        op0=Alu.mult,
                        op1=Alu.mult,
                    )
                    yo = ypool.tile([P, WIDTH], cdt, tag="yo")
                    if generic_affine:
                        t1 = ypool.tile([P, WIDTH], f32, tag="t1")
                        nc.scalar.activation(
                            out=t1[:],
                            in_=po[:, :WIDTH],
                            func=Act.Identity,
                            scale=rstd[:, 0:1],
                            bias=nb[:, 0:1],
                        )
                        t2 = ypool.tile([P, WIDTH], f32, tag="t2")
                        nc.vector.tensor_tensor(
                            out=t2[:], in0=t1[:], in1=gamma_sb, op=Alu.mult
                        )
                        t3 = ypool.tile([P, WIDTH], f32, tag="t3")
                        nc.vector.tensor_tensor(
                            out=t3[:], in0=t2[:], in1=beta_sb, op=Alu.add
                        )
                        nc.scalar.activation(out=yo[:], in_=t3[:], func=Act.Relu)
                    else:
                        nc.scalar.activation(
                            out=yo[:],
                            in_=po[:, :WIDTH],
                            func=Act.Relu,
                            scale=rstd[:, 0:1],
                            bias=nb[:, 0:1],
                        )
                    rows = min(P, NODES_PER_CORE - b * P)
                    nc.scalar.dma_start(out_d[b * P : b * P + rows, :], yo[:rows, :])
    return nc


def _pack_inputs(TL, TH, dstcol, dval, dinv, idxe, idxo, cnts, x, W, bias, gamma, beta, generic_affine):
    cnp = ml_dtypes.bfloat16 if USE_BF16 else np.float32
    Ttot = dstcol.shape[2]  # edge tiles + one self-loop tile per block

    xc = (x * dinv[:, None]).astype(cnp)  # fold dinv[src] into the tables
    xe = np.ascontiguousarray(xc[0::2])
    xo = np.ascontiguousarray(xc[1::2])
    WT32 = W.T.astype(np.float32)  # [in, out]
    rs = WT32.sum(axis=1, keepdims=True)  # [256, 1] row sums
    WTe = np.concatenate([WT32, rs], axis=1).astype(cnp)  # [256, 257]
    wt = np.ascontiguousarray(np.concatenate([WTe[:P], WTe[P:]], axis=1))  # [128, 514]
    b32 = bias.astype(np.float32)
    be = np.ascontiguousarray(
        np.concatenate([b32, [b32.sum()]]).astype(cnp)[None, :]
    )  # [1, 257]

    iota = np.arange(P, dtype=np.float32)
    in_maps = []
    for c in range(N_CORES):
        # S[e, t*128+d] = dinv[dst] if dstcol[e,t]==d else 0 (dinv[src] lives
        # in the gather tables; fp8 quantization of the per-dst scale washes
        # out in LayerNorm up to the tiny bias coupling)
        sm = (dstcol[c][:, :, None] == iota[None, None, :]) * dval[c][:, :, None]
        smat = np.ascontiguousarray(
            sm.reshape(P, Ttot * P).astype(ml_dtypes.float8_e4m3)
        )
        cc = np.ascontiguousarray(cnts[c][None, :])
        sl0 = c * (NODES_PER_CORE // 2)
        xsl = np.zeros((N_BLOCKS * P, WIDTH), cnp)
        for b2 in range(N_BLOCKS):
            rows = min(P, NODES_PER_CORE - b2 * P)
            nev = (rows + 1) // 2
            nod = rows // 2
            xsl[b2 * P : b2 * P + nev] = xe[sl0 + b2 * 64 : sl0 + b2 * 64 + nev]
            xsl[b2 * P + 64 : b2 * P + 64 + nod] = (
                xo[sl0 + b2 * 64 : sl0 + b2 * 64 + nod]
            )
        m = {
            "xe": xe,
            "xo": xo,
            "xsl": np.ascontiguousarray(xsl),
            "idxe": np.ascontiguousarray(idxe[c]),
            "idxo": np.ascontiguousarray(idxo[c]),
            "smat": smat,
            "wt": wt,
            "be": be,
            "cnts": cc,
        }
        if generic_affine:
            m["gb"] = np.ascontiguousarray(
                np.concatenate(
                    [
                        np.tile(gamma.astype(np.float32)[None, :], (P, 1)),
                        np.tile(beta.astype(np.float32)[None, :], (P, 1)),
                    ],
                    axis=1,
                )
            )
        in_maps.append(m)
    return in_maps


_PROGRAM_CACHE = {}


def kernel(x, edge_index, W, b, gamma, beta, _run_kwargs=None):
    from concourse.bass_utils import run_bass_kernel_spmd

    x = np.asarray(x)
    W = np.asarray(W)
    bias = np.asarray(b)
    gamma = np.asarray(gamma)
    beta = np.asarray(beta)

    TL, TH, dstcol, normv, dval, dinv, idxe, idxo, cnts = _preprocess(edge_index)
    generic_affine = not (np.all(gamma == 1.0) and np.all(beta == 0.0))

    key = (tuple(TL), tuple(TH), generic_affine)
    if key not in _PROGRAM_CACHE:
        nc = _build_program(TL, TH, generic_affine)
        nc.finalize()
        _PROGRAM_CACHE[key] = nc
    nc = _PROGRAM_CACHE[key]

    in_maps = _pack_inputs(
        TL, TH, dstcol, dval, dinv, idxe, idxo, cnts, x, W, bias, gamma, beta, generic_affine
    )

    kwargs = dict(_run_kwargs or {})
    kwargs.pop("_result", None)
    rr = run_bass_kernel_spmd(nc, in_maps, list(range(N_CORES)), **kwargs)
    out = np.concatenate([np.asarray(rr.results[c]["out"]) for c in range(N_CORES)], axis=0)
    if _run_kwargs is not None:
        _run_kwargs["_result"] = rr
    return np.ascontiguousarray(out.astype(np.float32))


# revision 33
# speedup vs baseline: 1.0339x; 1.0339x over previous
"""GCN block (GCNConv + LayerNorm + ReLU) on 8 Trainium2 NeuronCores.

Strategy (matches the "shard nodes / partition edges by destination" hint):
  - out = LN(A_norm @ (x @ W^T) + b) = LN((A_norm @ x) @ W^T + b): aggregate
    raw features first (A_norm commutes with the linear map), so the random
    gather runs on node-major x and no transposes are needed anywhere.
  - Destination nodes are sharded contiguously across the 8 cores
    (6250 rows each); each core processes the edges that point into its
    shard.  x is replicated in every core's DRAM as two bf16 gather tables
    (even/odd node rows, so row indices fit dma_gather's int16 indices);
    table rows are pre-scaled by dinv[src] on the host.
  - Edge messages are bucketed per (128-dst-block, src parity) and fetched
    with one dma_gather per bucket (98 small calls overlap desc-gen across
    the 4 SWDGE queues).  Buckets are padded with trailing -1 indices and
    the per-core TRUE count is loaded into a gpsimd register (batched 8 at
    a time), so padding slots generate no DMA descriptors.  Self-loop
    messages bypass the gather entirely: each block's own rows are one
    contiguous host-packed DMA (xsl) scattered via a permuted-diagonal S.
  - For each 128-message tile the [128e x 128d] selection matrix
    S (S[e, d] = dinv[dst] if dst_e == d) is PRECOMPUTED ON HOST in
    fp8-e4m3 and streamed in with plain contiguous DMA (building S on the
    DVE dominated the v1 trace; fp8 halves its footprint and LayerNorm
    cancels the per-dst-row quantization up to the tiny bias coupling).
    The scatter-add is G_blk^T @ S accumulated in PSUM, which directly
    yields agg^T as [channel, dst] — exactly the stationary operand the
    W-matmul wants.  agg^T @ W^T gives [dst, out_ch] node-major; the bias
    is folded in as a rank-1 matmul (ones^T @ [b | sum(b)]) into the same
    PSUM tile, and LayerNorm+ReLU run on ACT/DVE: the mean comes free from
    an extra W column (row-sums), E[y^2] from one Square+accum pass, and
    the finale is a single fused Relu(y*rstd - mu*rstd) with a bf16 store.
"""

import math
import sys

sys.path.insert(0, "/opt/trn_rl_repo")

import numpy as np
import ml_dtypes

N_NODES = 50000
WIDTH = 256
N_CORES = 8
NODES_PER_CORE = N_NODES // N_CORES  # 6250
P = 128
N_BLOCKS = math.ceil(NODES_PER_CORE / P)  # 49 (last block has 106 rows)
LN_EPS = 1e-5
HALF = N_NODES // 2  # rows per gather table

USE_BF16 = True
GATHER_TILE_CAP = 8  # max tiles (128 idxs each) per dma_gather call (HW ring limit 1024)


def _preprocess(edge_index):
    """Bucket messages by (core, dst-block, src-parity table), pad each bucket
    to whole 128-edge tiles.

    Processing tile order: per block, even-table tiles then odd-table tiles.
    Gather order: even tiles of all blocks concatenated (ditto odd).
    Returns (TL, TH, dstcol[8,P,Ttot], normv[8,P,Ttot],
             idxe[8,128,8*sum(TL)] i16, idxo[8,128,8*sum(TH)] i16).
    """
    src = np.asarray(edge_index[0]).astype(np.int64)
    dst = np.asarray(edge_index[1]).astype(np.int64)
    loops = np.arange(N_NODES, dtype=np.int64)
    # degree includes self-loops; the self-loop messages themselves bypass
    # the gather (contiguous rows, see the per-block diagonal S tile below)
    deg = np.bincount(np.concatenate([dst, loops]), minlength=N_NODES).astype(
        np.float64
    )
    dinv = 1.0 / np.sqrt(deg)  # deg >= 1 thanks to self loops
    msrc = src
    mdst = dst
    norm = (dinv[msrc] * dinv[mdst]).astype(np.float32)

    core = mdst // NODES_PER_CORE
    r = mdst % NODES_PER_CORE
    blk = np.minimum(r // P, N_BLOCKS - 1)
    dcol = (r - blk * P).astype(np.float32)
    tab = msrc & 1
    gbin = (core * N_BLOCKS + blk) * 2 + tab

    order = np.argsort(gbin, kind="stable")
    msrc, norm, dcol, gbin = msrc[order], norm[order], dcol[order], gbin[order]

    cnt = np.bincount(gbin, minlength=N_CORES * N_BLOCKS * 2).reshape(
        N_CORES, N_BLOCKS, 2
    )
    TL = [int(math.ceil(int(cnt[:, b, 0].max()) / P)) for b in range(N_BLOCKS)]
    TH = [int(math.ceil(int(cnt[:, b, 1].max()) / P)) for b in range(N_BLOCKS)]
    sTL, sTH = sum(TL), sum(TH)
    Ttot = sTL + sTH + N_BLOCKS  # +1 self-loop tile per block
    # tile offsets
    EOFF = np.concatenate([[0], np.cumsum(TL)])  # even gather order
    OOFF = np.concatenate([[0], np.cumsum(TH)])  # odd gather order
    TOFF = np.concatenate(
        [[0], np.cumsum(np.asarray(TL) + np.asarray(TH) + 1)]
    )

    dstcol = np.zeros((N_CORES, P, Ttot), np.float32)
    normv = np.zeros((N_CORES, P, Ttot), np.float32)
    dval = np.zeros((N_CORES, P, Ttot), np.float32)
    idxe_flat = np.full((N_CORES, sTL * P), -1, np.int16)
    idxo_flat = np.full((N_CORES, sTH * P), -1, np.int16)

    starts = np.concatenate([[0], np.cumsum(cnt.ravel())])[:-1]
    j = np.arange(len(gbin)) - starts[gbin]  # index within bucket
    c = gbin // (N_BLOCKS * 2)
    b = (gbin // 2) % N_BLOCKS
    t = gbin & 1
    tile_in_bucket = j // P
    p = j % P
    # metadata in processing order
    tg = np.where(
        t == 0,
        TOFF[b] + tile_in_bucket,
        TOFF[b] + np.asarray(TL)[b] + tile_in_bucket,
    )
    dstcol[c, p, tg] = dcol
    normv[c, p, tg] = norm
    dval[c, p, tg] = dinv[mdst[order]].astype(np.float32)
    # self-loop tiles: partition p<64 -> local dst 2p (even global parity),
    # p>=64 -> local dst 2(p-64)+1; value dinv[global dst]
    for c2 in range(N_CORES):
        for b2 in range(N_BLOCKS):
            tgs = TOFF[b2] + TL[b2] + TH[b2]
            base = c2 * NODES_PER_CORE + b2 * P
            rows = min(P, NODES_PER_CORE - b2 * P)
            nev = (rows + 1) // 2
            nod = rows // 2
            pe_ = np.arange(nev)
            dstcol[c2, pe_, tgs] = 2 * pe_
            dval[c2, pe_, tgs] = dinv[base + 2 * pe_]
            po_ = np.arange(nod)
            dstcol[c2, 64 + po_, tgs] = 2 * po_ + 1
            dval[c2, 64 + po_, tgs] = dinv[base + 2 * po_ + 1]
    # gather index arrays (per-table tile order)
    idx16 = (msrc >> 1).astype(np.int16)
    Je = (EOFF[b] + tile_in_bucket) * P + p
    Jo = (OOFF[b] + tile_in_bucket) * P + p
    ev = t == 0
    idxe_flat[c[ev], Je[ev]] = idx16[ev]
    idxo_flat[c[~ev], Jo[~ev]] = idx16[~ev]

    # merged gather calls: pairs of adjacent blocks share one call per parity.
    # The first block's pad slots must hold a VALID index (0) since only
    # trailing negatives are elided; the second block's pads stay -1.
    pairs = _pairs(TL, TH)
    ncalls = len(pairs) * 2
    ccnt = np.zeros((N_CORES, ncalls), np.int32)
    for pi, pr in enumerate(pairs):
        a = pr[0]
        if len(pr) == 2:
            b2 = pr[1]
            # zero-fill block a's pads in both parity streams
            for c2 in range(N_CORES):
                e_lo = EOFF[a] * P + cnt[c2, a, 0]
                idxe_flat[c2, e_lo : (EOFF[a] + TL[a]) * P] = 0
                o_lo = OOFF[a] * P + cnt[c2, a, 1]
                idxo_flat[c2, o_lo : (OOFF[a] + TH[a]) * P] = 0
            ccnt[:, 2 * pi] = TL[a] * P + cnt[:, b2, 0]
            ccnt[:, 2 * pi + 1] = TH[a] * P + cnt[:, b2, 1]
        else:
            ccnt[:, 2 * pi] = cnt[:, a, 0]
            ccnt[:, 2 * pi + 1] = cnt[:, a, 1]

    # wrap: flat j -> (partition j%16, column j//16), replicated on 8 stripes
    def wrap(flat, ntiles):
        if ntiles == 0:
            return np.zeros((N_CORES, P, 0), np.int16)
        a = flat.reshape(N_CORES, ntiles * 8, 16).transpose(0, 2, 1)  # [8,16,cols]
        return np.ascontiguousarray(np.tile(a, (1, 8, 1)))  # [8,128,cols]

    return (TL, TH, dstcol, normv, dval, dinv.astype(np.float32),
            wrap(idxe_flat, sTL), wrap(idxo_flat, sTH), ccnt)


def _pairs(TL, TH):
    """Greedy pairing of adjacent blocks into merged gather calls; each
    parity's tile total must stay within the 1024-descriptor ring cap."""
    return [(b,) for b in range(N_BLOCKS)]


def _chunks(TL, TH):
    return [
        (
            list(pr),
            sum(TL[b] for b in pr),
            sum(TH[b] for b in pr),
        )
        for pr in _pairs(TL, TH)
    ]


def _build_program(TL, TH, generic_affine):
    import concourse.bass as bass
    import concourse.tile as tile
    from concourse import bacc as bacc_mod
    from concourse import mybir
    from contextlib import ExitStack

    f32 = mybir.dt.float32
    bf16 = mybir.dt.bfloat16
    cdt = bf16 if USE_BF16 else f32
    i16 = mybir.dt.int16
    Alu = mybir.AluOpType
    Act = mybir.ActivationFunctionType
    sTL, sTH = sum(TL), sum(TH)
    Ttot = sTL + sTH + N_BLOCKS
    EOFF = np.concatenate([[0], np.cumsum(TL)])
    OOFF = np.concatenate([[0], np.cumsum(TH)])
    TOFF = np.concatenate([[0], np.cumsum(np.asarray(TL) + np.asarray(TH) + 1)])
    chunks = _chunks(TL, TH)
    max_nt = max(ch[1] + ch[2] for ch in chunks)

    nc = bacc_mod.Bacc(None, target_bir_lowering=False, debug=False, num_swdge_queues=4)
    xe_d = nc.declare_dram_parameter("xe", [HALF, WIDTH], cdt, isOutput=False)
    xo_d = nc.declare_dram_parameter("xo", [HALF, WIDTH], cdt, isOutput=False)
    xsl_d = nc.declare_dram_parameter("xsl", [N_BLOCKS * P, WIDTH], cdt, isOutput=False)
    idxe_d = nc.declare_dram_parameter("idxe", [P, 8 * sTL], i16, isOutput=False)
    idxo_d = nc.declare_dram_parameter("idxo", [P, 8 * sTH], i16, isOutput=False)
    f8 = mybir.dt.float8e4
    smat_d = nc.declare_dram_parameter("smat", [P, Ttot * P], f8, isOutput=False)
    wt_d = nc.declare_dram_parameter("wt", [P, 2 * (WIDTH + 1)], cdt, isOutput=False)
    be_d = nc.declare_dram_parameter("be", [1, WIDTH + 1], cdt, isOutput=False)
    i32 = mybir.dt.int32
    NCALLS = 2 * len(_pairs(TL, TH))
    cnts_d = nc.declare_dram_parameter("cnts", [1, NCALLS], i32, isOutput=False)
    if generic_affine:
        gb_d = nc.declare_dram_parameter("gb", [P, 2 * WIDTH], f32, isOutput=False)
    out_d = nc.declare_dram_parameter("out", [NODES_PER_CORE, WIDTH], cdt, isOutput=True)

    with tile.TileContext(nc) as tc:
        with ExitStack() as ctx:
            const = ctx.enter_context(tc.tile_pool(name="const", bufs=1))
            gpool = ctx.enter_context(tc.tile_pool(name="g", bufs=8))
            gspool = ctx.enter_context(tc.tile_pool(name="gs", bufs=4))
            spool = ctx.enter_context(tc.tile_pool(name="s", bufs=4))
            apool = ctx.enter_context(tc.tile_pool(name="aggT", bufs=2))
            ypool = ctx.enter_context(tc.tile_pool(name="y", bufs=3))
            stat = ctx.enter_context(tc.tile_pool(name="stat", bufs=4))
            ppool = ctx.enter_context(tc.tile_pool(name="psA", bufs=3, space="PSUM"))
            opsum = ctx.enter_context(tc.tile_pool(name="psO", bufs=2, space="PSUM"))

            cnts_sb = const.tile([1, NCALLS], i32)
            nc.sync.dma_start(cnts_sb[:], cnts_d[:, :])
            idxe_sb = const.tile([P, 8 * sTL], i16)
            nc.sync.dma_start(idxe_sb[:], idxe_d[:, :])
            idxo_sb = const.tile([P, 8 * sTH], i16)
            nc.sync.dma_start(idxo_sb[:], idxo_d[:, :])
            wt_sb = const.tile([P, 2 * (WIDTH + 1)], cdt)
            nc.sync.dma_start(wt_sb[:], wt_d[:, :])
            be_sb = const.tile([1, WIDTH + 1], cdt)
            nc.sync.dma_start(be_sb[:], be_d[:, :])
            ones_sb = const.tile([1, P], cdt)
            nc.vector.memset(ones_sb[:], 1.0)
            eps_sb = const.tile([P, 1], f32)
            nc.vector.memset(eps_sb[:], LN_EPS)
            if generic_affine:
                gb_sb = const.tile([P, 2 * WIDTH], f32)
                nc.sync.dma_start(gb_sb[:], gb_d[:, :])
                gamma_sb = gb_sb[:, 0:WIDTH]
                beta_sb = gb_sb[:, WIDTH : 2 * WIDTH]

            # warm up the Q7 gather path while the real idx tables load
            wi = const.tile([P, 8], i16)
            nc.vector.memset(wi[:], 0)
            wg = const.tile([P, 1, WIDTH], cdt)
            nc.gpsimd.dma_gather(
                wg[:], xe_d[:, :], wi[:, :], P, P, WIDTH, queue_num=3
            )

            qn = 0
            gregs = [nc.gpsimd.alloc_register(f"gcnt{i}") for i in range(8)]
            GP_BUFS = 8
            # zero all gather buffers once: rows beyond a bucket's true count
            # are never written by the gather and must not be NaN (S is 0
            # there, but 0*NaN would poison the PSUM accumulation)
            maxTL = max(ch[1] for ch in chunks)
            maxTH = max(ch[2] for ch in chunks)
            for _ in range(GP_BUFS):
                zt = gpool.tile([P, maxTL, WIDTH], cdt, tag="ge")
                nc.vector.memset(zt[:], 0.0)
                zt2 = gpool.tile([P, maxTH, WIDTH], cdt, tag="go")
                nc.vector.memset(zt2[:], 0.0)

            def bucket_gather(tag, ntiles, idx_sb, off, x_d, cidx):
                if ntiles == 0:
                    return None
                nonlocal qn
                gt = gpool.tile([P, ntiles, WIDTH], cdt, tag=tag)
                assert cidx == qn, (cidx, qn)  # batched reg loads rely on this
                if qn % 8 == 0:
                    hi = min(qn + 8, NCALLS)
                    nc.gpsimd.reg_load(gregs[: hi - qn], cnts_sb[0:1, qn:hi])
                reg = gregs[qn % 8]
                nc.gpsimd.dma_gather(
                    gt[:],
                    x_d[:, :],
                    idx_sb[:, 8 * off : 8 * (off + ntiles)],
                    ntiles * P,
                    reg,
                    WIDTH,
                    single_packet=False,
                    queue_num=qn % 4,
                )
                qn += 1
                return gt

            for pi, (blocks, ne, no) in enumerate(chunks):
                tgc0 = int(TOFF[blocks[0]])
                nt_chunk = ne + no + len(blocks)
                s_sb = spool.tile([P, nt_chunk * P], f8, tag="schunk")
                nc.sync.dma_start(s_sb[:], smat_d[:, tgc0 * P : (tgc0 + nt_chunk) * P])
                a = blocks[0]
                ge_m = bucket_gather("ge", ne, idxe_sb, int(EOFF[a]), xe_d, 2 * pi)
                go_m = bucket_gather("go", no, idxo_sb, int(OOFF[a]), xo_d, 2 * pi + 1)
                gstiles = {}
                for b in blocks:
                    # self-loop rows, host-packed per block: partitions 0-63 =
                    # even-parity dsts, 64-127 = odd (zeros in pad rows)
                    gs = gspool.tile([P, 1, WIDTH], cdt, tag="gs")
                    nc.sync.dma_start(gs[:, 0, :], xsl_d[b * P : (b + 1) * P, :])
                    gstiles[b] = gs
                for b in blocks:
                    tg0 = int(TOFF[b])
                    gs = gstiles[b]
                    eoff = int(EOFF[b] - EOFF[a])
                    ooff = int(OOFF[b] - OOFF[a])
                    seq = [(ge_m, eoff + t) for t in range(TL[b])] + [
                        (go_m, ooff + t) for t in range(TH[b])
                    ] + [(gs, 0)]
                    nt = len(seq)
                    ps0_t = ppool.tile([P, P], f32, tag="ps0")
                    ps1_t = ppool.tile([P, P], f32, tag="ps1")
                    ps0 = ps0_t[:]
                    ps1 = ps1_t[:]
                    for k, (gt, col) in enumerate(seq):
                        so = (tg0 - tgc0 + k) * P
                        s_ap = s_sb[:, so : so + P]
                        nc.tensor.matmul(
                            out=ps0,
                            lhsT=gt[:, col, 0:P],
                            rhs=s_ap,
                            start=(k == 0),
                            stop=(k == nt - 1),
                            skip_group_check=True,
                        )
                        nc.tensor.matmul(
                            out=ps1,
                            lhsT=gt[:, col, P:WIDTH],
                            rhs=s_ap,
                            start=(k == 0),
                            stop=(k == nt - 1),
                            skip_group_check=True,
                        )
                    # aggT blocks [128 ch, 128 dst] -> SBUF (cast) for W-matmul
                    a0 = apool.tile([P, P], cdt, tag="a0")
                    nc.scalar.copy(a0[:], ps0)
                    a1 = apool.tile([P, P], cdt, tag="a1")
                    nc.scalar.copy(a1[:], ps1)
                    po = opsum.tile([P, WIDTH + 1], f32, tag="po")
                    nc.tensor.matmul(
                        out=po[:],
                        lhsT=a0[:],
                        rhs=wt_sb[:, : WIDTH + 1],
                        start=True,
                        stop=False,
                    )
                    nc.tensor.matmul(
                        out=po[:],
                        lhsT=a1[:],
                        rhs=wt_sb[:, WIDTH + 1 :],
                        start=False,
                        stop=False,
                    )
                    # rank-1 bias add: po += ones^T @ [b | sum(b)]
                    nc.tensor.matmul(
                        out=po[:],
                        lhsT=ones_sb[:, :],
                        rhs=be_sb[:, :],
                        start=False,
                        stop=True,
                        skip_group_check=True,
                    )
                    # ---- epilogue: LayerNorm stats + fused scale/ReLU ----
                    # po[:, :256] == y (bias already added); po[:, 256] == 256*mean(y)
                    sq = ypool.tile([P, WIDTH], cdt, tag="sq")
                    ey2 = stat.tile([P, 1], f32, tag="ey2")
                    nc.scalar.activation(
                        out=sq[:],
                        in_=po[:, :WIDTH],
                        func=Act.Square,
                        scale=1.0 / 16.0,
                        accum_out=ey2[:],
                    )
                    mu = stat.tile([P, 1], f32, tag="mu")
                    nc.vector.tensor_scalar_mul(mu[:], po[:, WIDTH : WIDTH + 1], 1.0 / WIDTH)
                    m2 = stat.tile([P, 1], f32, tag="m2")
                    nc.vector.tensor_tensor(m2[:], mu[:], mu[:, 0:1], Alu.mult)
                    var = stat.tile([P, 1], f32, tag="var")
                    nc.vector.tensor_scalar_sub(var[:], ey2[:], m2[:, 0:1])
                    sd = stat.tile([P, 1], f32, tag="sd")
                    nc.scalar.activation(
                        out=sd[:], in_=var[:], func=Act.Sqrt, bias=eps_sb[:, :1]
                    )
                    rstd = stat.tile([P, 1], f32, tag="rstd")
                    nc.vector.reciprocal(rstd[:], sd[:])
                    nb = stat.tile([P, 1], f32, tag="nb")
                    nc.vector.tensor_scalar(
                        out=nb[:],
                        in0=mu[:],
                        scalar1=rstd[:, 0:1],
                        scalar2=-1.0,
                        op0=Alu.mult,
                        op1=Alu.mult,
                    )
                    yo = ypool.tile([P, WIDTH], cdt, tag="yo")
                    if generic_affine:
                        t1 = ypool.tile([P, WIDTH], f32, tag="t1")
                        nc.scalar.activation(
                            out=t1[:],
                            in_=po[:, :WIDTH],
                            func=Act.Identity,
                            scale=rstd[:, 0:1],
                            bias=nb[:, 0:1],
                        )
                        t2 = ypool.tile([P, WIDTH], f32, tag="t2")
                        nc.vector.tensor_tensor(
                            out=t2[:], in0=t1[:], in1=gamma_sb, op=Alu.mult
                        )
                        t3 = ypool.tile([P, WIDTH], f32, tag="t3")
                        nc.vector.tensor_tensor(
                            out=t3[:], in0=t2[:], in1=beta_sb, op=Alu.add
                        )
                        nc.scalar.activation(out=yo[:], in_=t3[:], func=Act.Relu)
                    else:
                        nc.scalar.activation(
                            out=yo[:],
                            in_=po[:, :WIDTH],
                            func=Act.Relu,
                            scale=rstd[:, 0:1],
                            bias=nb[:, 0:1],
                        )
                    rows = min(P, NODES_PER_CORE - b * P)
                    nc.sync.dma_start(out_d[b * P : b * P + rows, :], yo[:rows, :])
    return nc


def _pack_inputs(TL, TH, dstcol, dval, dinv, idxe, idxo, cnts, x, W, bias, gamma, beta, generic_affine):
    cnp = ml_dtypes.bfloat16 if USE_BF16 else np.float32
    Ttot = dstcol.shape[2]  # edge tiles + one self-loop tile per block

    xc = (x * dinv[:, None]).astype(cnp)  # fold dinv[src] into the tables
    xe = np.ascontiguousarray(xc[0::2])
    xo = np.ascontiguousarray(xc[1::2])
    WT32 = W.T.astype(np.float32)  # [in, out]
    rs = WT32.sum(axis=1, keepdims=True)  # [256, 1] row sums
    WTe = np.concatenate([WT32, rs], axis=1).astype(cnp)  # [256, 257]
    wt = np.ascontiguousarray(np.concatenate([WTe[:P], WTe[P:]], axis=1))  # [128, 514]
    b32 = bias.astype(np.float32)
    be = np.ascontiguousarray(
        np.concatenate([b32, [b32.sum()]]).astype(cnp)[None, :]
    )  # [1, 257]

    iota = np.arange(P, dtype=np.float32)
    in_maps = []
    for c in range(N_CORES):
        # S[e, t*128+d] = dinv[dst] if dstcol[e,t]==d else 0 (dinv[src] lives
        # in the gather tables; fp8 quantization of the per-dst scale washes
        # out in LayerNorm up to the tiny bias coupling)
        sm = (dstcol[c][:, :, None] == iota[None, None, :]) * dval[c][:, :, None]
        smat = np.ascontiguousarray(
            sm.reshape(P, Ttot * P).astype(ml_dtypes.float8_e4m3)
        )
        cc = np.ascontiguousarray(cnts[c][None, :])
        sl0 = c * (NODES_PER_CORE // 2)
        xsl = np.zeros((N_BLOCKS * P, WIDTH), cnp)
        for b2 in range(N_BLOCKS):
            rows = min(P, NODES_PER_CORE - b2 * P)
            nev = (rows + 1) // 2
            nod = rows // 2
            xsl[b2 * P : b2 * P + nev] = xe[sl0 + b2 * 64 : sl0 + b2 * 64 + nev]
            xsl[b2 * P + 64 : b2 * P + 64 + nod] = (
                xo[sl0 + b2 * 64 : sl0 + b2 * 64 + nod]
            )
        m = {
            "xe": xe,
            "xo": xo,
            "xsl": np.ascontiguousarray(xsl),
            "idxe": np.ascontiguousarray(idxe[c]),
            "idxo": np.ascontiguousarray(idxo[c]),
            "smat": smat,
            "wt": wt,
            "be": be,
            "cnts": cc,
        }
        if generic_affine:
            m["gb"] = np.ascontiguousarray(
                np.concatenate(
                    [
                        np.tile(gamma.astype(np.float32)[None, :], (P, 1)),
                        np.tile(beta.astype(np.float32)[None, :], (P, 1)),
                    ],
                    axis=1,
                )
            )
        in_maps.append(m)
    return in_maps


_PROGRAM_CACHE = {}


def kernel(x, edge_index, W, b, gamma, beta, _run_kwargs=None):
    from concourse.bass_utils import run_bass_kernel_spmd

    x = np.asarray(x)
    W = np.asarray(W)
    bias = np.asarray(b)
    gamma = np.asarray(gamma)
    beta = np.asarray(beta)

    TL, TH, dstcol, normv, dval, dinv, idxe, idxo, cnts = _preprocess(edge_index)
    generic_affine = not (np.all(gamma == 1.0) and np.all(beta == 0.0))

    key = (tuple(TL), tuple(TH), generic_affine)
    if key not in _PROGRAM_CACHE:
        nc = _build_program(TL, TH, generic_affine)
        nc.finalize()
        _PROGRAM_CACHE[key] = nc
    nc = _PROGRAM_CACHE[key]

    in_maps = _pack_inputs(
        TL, TH, dstcol, dval, dinv, idxe, idxo, cnts, x, W, bias, gamma, beta, generic_affine
    )

    kwargs = dict(_run_kwargs or {})
    kwargs.pop("_result", None)
    rr = run_bass_kernel_spmd(nc, in_maps, list(range(N_CORES)), **kwargs)
    out = np.concatenate([np.asarray(rr.results[c]["out"]) for c in range(N_CORES)], axis=0)
    if _run_kwargs is not None:
        _run_kwargs["_result"] = rr
    return np.ascontiguousarray(out.astype(np.float32))
